# revision 6
# baseline (speedup 1.0000x reference)
"""JointAtt (dense_cnn) Trainium2 Bass kernel — v4 (GpSimd-free, 4-slice batch).

Per core: 8 slices (n,g) of x [128, 4096] fp16, processed as 2 groups of 4.
Group layout: slice i of a group owns partition band b=32i (PE matmul
tile_position cols {0,32,64,96}), so the pooling octaves of all 4 slices
live in ONE PSUM tile P4 [128, 2, 64, 8] and are folded by a single DVE
tensor_reduce — no GpSimd trees (which contended with the DVE's SBUF
ports and serialized the whole kernel in v3).

  PE:     per slice 16 accumulating matmuls (Yh octaves h-major, Yw octaves
          w-major so both fold over the innermost axis); 2 attention matmuls
          per slice from partition band b (whw replicated per band).
  DVE:    1 tensor_reduce fold per group (PSUM->SBUF, [128,2,64,8]->[128,128]);
          hswish smalls batched [128,128]; per slice 2 big fp16 2x-rate
          tensor_tensor multiplies OUT = X * ahe * aw.
  Scalar: batched Relu (hswish), batched sigmoids (AHE4 with broadcast width-2
          trick, AW4); store DMA triggers.
  DMA:    1 contiguous 1 MB load (sync ring) + 1 MB store (scalar ring) per
          slice; channel shuffle + fp32 conversion on the host.
"""

import numpy as np

import concourse.bass as bass
import concourse.bacc as bacc
import concourse.mybir as mybir
import concourse.tile as tile
from concourse.bass_utils import run_bass_kernel_spmd

F32 = mybir.dt.float32
F16 = mybir.dt.float16

N_CORES = 8
NB = 2          # batches per core
C = 512
G = 4           # groups (of channels, in the model)
CG = 128        # channels per group
H = 64
W = 64
HW = H * W
S = NB * G      # slices per core
GRP = 4         # slices per partition-batched group
MIP = 16        # conv1 output channels
J = 8           # pooling octave width
EPS = 1e-5

_NC_CACHE = None


def _build_bass():
    nc = bacc.Bacc(None, target_bir_lowering=False)

    x_d = nc.dram_tensor("x", [S, CG, HW], F16, kind="ExternalInput")
    w1t_d = nc.dram_tensor("w1t", [CG, MIP], F16, kind="ExternalInput")
    whw4_d = nc.dram_tensor("whw4", [CG, 2 * CG], F16, kind="ExternalInput")
    bact4_d = nc.dram_tensor("bact4", [CG, 1], F32, kind="ExternalInput")
    bhw_d = nc.dram_tensor("bhw", [CG, 2], F32, kind="ExternalInput")
    out_d = nc.dram_tensor("out", [S, CG, HW], F16, kind="ExternalOutput")

    Relu = mybir.ActivationFunctionType.Relu
    Sigmoid = mybir.ActivationFunctionType.Sigmoid
    ADD = mybir.AluOpType.add
    MIN = mybir.AluOpType.min
    MULT = mybir.AluOpType.mult

    with tile.TileContext(nc) as tc:
        with (
            tc.tile_pool(name="consts", bufs=1) as consts,
            tc.tile_pool(name="xp", bufs=6) as xp,
            tc.tile_pool(name="op", bufs=4) as op,
            tc.tile_pool(name="ps", bufs=1, space="PSUM") as ps,
            tc.tile_pool(name="sm", bufs=2) as sm,
        ):
            w1t = consts.tile([CG, MIP], F16)
            nc.scalar.dma_start(out=w1t, in_=w1t_d[:])
            whw4 = consts.tile([CG, 2 * CG], F16)
            nc.scalar.dma_start(out=whw4, in_=whw4_d[:])
            bact4 = consts.tile([CG, 1], F32)
            nc.scalar.dma_start(out=bact4, in_=bact4_d[:])
            bhw = consts.tile([CG, 2], F32)
            nc.scalar.dma_start(out=bhw, in_=bhw_d[:])
            bh = bhw[:, 0:1]
            bw = bhw[:, 1:2]

            for g in range(S // GRP):
                # ---- conv1+pooling octaves for 4 slices into one PSUM tile.
                # band b=32i: Yh octaves at [b:b+16, 0, h, j] (j = w octave),
                # Yw octaves at [b:b+16, 1, w, j] (j = h octave, w-major so
                # both directions fold over the innermost axis).
                P4 = ps.tile([CG, 2, H, J], F32, name="P4", tag="P4", bufs=2)
                Xs = []
                for i in range(GRP):
                    s = GRP * g + i
                    b = 32 * i
                    X = xp.tile([CG, HW], F16, name="X")
                    nc.sync.dma_start(out=X, in_=x_d[s])
                    Xs.append(X)
                    Xr = X.rearrange("p (h w) -> p h w", h=H)
                    for k in range(HW // (H * J)):
                        nc.tensor.matmul(
                            P4[b : b + MIP, 0:1, :, :],
                            w1t,
                            Xr[:, :, J * k : J * (k + 1)],
                            start=(k == 0),
                            stop=(k == HW // (H * J) - 1),
                            tile_position=(0, b),
                        )
                    for k in range(HW // (W * J)):
                        Xw = X[:, J * W * k : J * W * (k + 1)].rearrange(
                            "p (j w) -> p w j", j=J
                        )
                        nc.tensor.matmul(
                            P4[b : b + MIP, 1:2, :, :],
                            w1t,
                            Xw,
                            start=(k == 0),
                            stop=(k == HW // (W * J) - 1),
                            tile_position=(0, b),
                        )

                # ---- skinny chain at high priority: fold + hswish + attention
                with tc.high_priority():
                    Y4 = sm.tile([CG, 2, H], F32, name="Y4", tag="Y4")
                    nc.vector.tensor_reduce(
                        out=Y4, in_=P4, axis=mybir.AxisListType.X, op=ADD
                    )
                    # hswish: HS = min(T,6) * (T-3), T = relu(Y + b1eff + 3)
                    T4 = sm.tile([CG, 2, H], F32, name="T4", tag="T4")
                    nc.scalar.activation(out=T4, in_=Y4, func=Relu, bias=bact4)
                    T3 = sm.tile([CG, 2, H], F32, name="T3", tag="T3")
                    nc.vector.tensor_scalar_add(out=T3, in0=T4, scalar1=-3.0)
                    HS4 = sm.tile([CG, 2, H], F16, name="HS4", tag="HS4")
                    nc.vector.scalar_tensor_tensor(
                        out=HS4, in0=T4, scalar=6.0, in1=T3, op0=MIN, op1=MULT
                    )

                    # attention logits: per slice, K=16 contraction on band b.
                    # Row-tiled matmuls must not share a PSUM bank (HW hazard),
                    # so slice i's logits live in their own 2KB bank:
                    # APs4[:, i, 0:128] with a 512-f32 (one bank) slice stride.
                    APs4 = ps.tile([CG, GRP, 512], F32, name="APs4", tag="APs4", bufs=1)
                    for i in range(GRP):
                        b = 32 * i
                        nc.tensor.matmul(
                            APs4[:, i : i + 1, 0:H],
                            whw4[b : b + MIP, 0:CG],
                            HS4[b : b + MIP, 0:1, :],
                            start=True,
                            stop=True,
                            tile_position=(b, 0),
                        )
                        nc.tensor.matmul(
                            APs4[:, i : i + 1, H : 2 * H],
                            whw4[b : b + MIP, CG:],
                            HS4[b : b + MIP, 1:2, :],
                            start=True,
                            stop=True,
                            tile_position=(b, 0),
                        )

                    # batched sigmoids; AHE4 materialized at width TWO only
                    # (stride-0 middle dim keeps the DVE multiply at 2x rate)
                    AHE4 = sm.tile([CG, GRP, H, 2], F16, name="AHE4", tag="AHE4")
                    nc.scalar.activation(
                        out=AHE4,
                        in_=APs4[:, :, 0:H].unsqueeze(3).broadcast_to([CG, GRP, H, 2]),
                        func=Sigmoid,
                        bias=bh,
                    )
                    AW4 = sm.tile([CG, GRP, W], F16, name="AW4", tag="AW4")
                    nc.scalar.activation(
                        out=AW4, in_=APs4[:, :, H : 2 * H], func=Sigmoid, bias=bw
                    )

                # ---- out = x * a_h_exp * a_w  (both 2x-rate on DVE)
                for i in range(GRP):
                    s = GRP * g + i
                    X = Xs[i]
                    OUT = op.tile([CG, HW], F16, name="OUT")
                    OUTr = OUT.rearrange("p (h w) -> p h w", h=H)
                    Xr4 = X.rearrange("p (h r two) -> p h r two", h=H, two=2)
                    OUTr4 = OUT.rearrange("p (h r two) -> p h r two", h=H, two=2)
                    ahe_b = AHE4[:, i].unsqueeze(2).broadcast_to([CG, H, W // 2, 2])
                    aw_b = AW4[:, i].unsqueeze(1).broadcast_to([CG, H, W])
                    nc.vector.tensor_tensor(out=OUTr4, in0=Xr4, in1=ahe_b, op=MULT)
                    nc.vector.tensor_tensor(out=OUTr, in0=OUTr, in1=aw_b, op=MULT)
                    nc.scalar.dma_start(out=out_d[s], in_=OUT)

    nc.finalize()
    return nc


def _get_nc():
    global _NC_CACHE
    if _NC_CACHE is None:
        _NC_CACHE = _build_bass()
    return _NC_CACHE


def _prep_weights(W1, b1, gamma, beta, mean, var, Wh, bh, Ww, bw):
    W1 = np.asarray(W1, np.float64)
    b1 = np.asarray(b1, np.float64)
    gamma = np.asarray(gamma, np.float64)
    beta = np.asarray(beta, np.float64)
    mean = np.asarray(mean, np.float64)
    var = np.asarray(var, np.float64)
    Wh = np.asarray(Wh, np.float64)
    Ww = np.asarray(Ww, np.float64)
    bh = np.asarray(bh, np.float64)
    bw = np.asarray(bw, np.float64)

    scale = gamma / np.sqrt(var + EPS)                    # (MIP,)
    w1eff = (W1 * scale[:, None]) / float(W)              # (MIP, CG); mean 1/64
    b1eff = scale * (b1 - mean) + beta                    # (MIP,)

    w1t = np.ascontiguousarray(w1eff.T.astype(np.float16))            # (CG, MIP)
    whw = np.concatenate([(Wh / 6.0).T, (Ww / 6.0).T], axis=1)        # (MIP, 2CG)
    whw4 = np.zeros((CG, 2 * CG), np.float16)
    bact4 = np.zeros((CG, 1), np.float32)
    for i in range(GRP):
        b = 32 * i
        whw4[b : b + MIP] = whw.astype(np.float16)
        bact4[b : b + MIP, 0] = (b1eff + 3.0).astype(np.float32)
    bhw = np.ascontiguousarray(
        np.stack([bh, bw], axis=1).astype(np.float32)
    )                                                     # (CG, 2)
    return w1t, whw4, bact4, bhw


def run(inputs: dict, trace: bool = False):
    """Run on 8 NeuronCores. Returns (out [16,512,64,64] fp32, results)."""
    x = np.asarray(inputs["x"], dtype=np.float32)
    n = x.shape[0]
    assert x.shape == (n, C, H, W) and n == N_CORES * NB, x.shape

    w1t, whw4, bact4, bhw = _prep_weights(
        inputs["W1"], inputs["b1"], inputs["gamma"], inputs["beta"],
        inputs["mean"], inputs["var"], inputs["Wh"], inputs["bh"],
        inputs["Ww"], inputs["bw"],
    )

    # fp16, pre-sliced per core: [core, slice(b,g), 128, 4096]
    x16 = np.ascontiguousarray(
        x.astype(np.float16).reshape(N_CORES, S, CG, HW)
    )

    nc = _get_nc()
    core_ids = list(range(N_CORES))
    in_maps = []
    for k in core_ids:
        in_maps.append(
            {
                "x": x16[k],
                "w1t": w1t,
                "whw4": whw4,
                "bact4": bact4,
                "bhw": bhw,
            }
        )

    res = run_bass_kernel_spmd(nc, in_maps, core_ids, trace=trace)
    out16 = np.stack([res.results[k]["out"] for k in core_ids])  # (8,8,128,HW)
    # group-major == natural channel order; then apply the channel shuffle
    # c' = (c % 4) * 128 + c // 4 on the host, with the fp16->fp32 upcast.
    nat = out16.astype(np.float32).reshape(n, C, H, W)
    out = np.ascontiguousarray(
        nat.reshape(n, CG, G, H, W).transpose(0, 2, 1, 3, 4).reshape(n, C, H, W)
    )
    return out, res


def kernel(**inputs) -> np.ndarray:
    out, _ = run(inputs, trace=False)
    return out


def exec_time_ns(res):
    return res.exec_time_ns


# revision 8
# speedup vs baseline: 1.2345x; 1.2345x over previous
"""JointAtt (dense_cnn) Trainium2 Bass kernel — v5 (GpSimd-free, 2-slice batch).

Per core: 8 slices (n,g) of x [128, 4096] fp16, processed as 4 groups of 2.
Group layout: slice i of a group owns partition band b=64i (PE matmul
tile_position cols {0,64}), so the pooling octaves of both slices live in
ONE PSUM tile P2 [128, 2, 512] and are folded by two DVE tensor_reduce ops
— no GpSimd trees (v3's trees contended with the DVE's SBUF ports and
serialized the whole kernel).

  PE:     ~3.4us of warmup matmuls while the first x load is in flight
          (HAM un-throttle: cold PE runs at 1.2 GHz, warm at 2.4);
          per slice 16 accumulating conv matmuls, all with contiguous or
          j-inner moving APs (216 ns each warm; a w-major moving AP would
          make consecutive columns 128B apart and halve the stream rate);
          2 attention matmuls per slice on row-tile b (whw replicated per
          band; each slice's logits in their OWN PSUM bank — concurrent
          row tiles sharing a bank is a HW hazard).
  DVE:    2 tensor_reduce folds per group (PSUM->SBUF, FD=512, the w-fold
          via a strided view — 1x mode doesn't care); 1 hswish STT per
          group; per slice 2 big fp16 2x-rate TTs OUT = X * ahe * aw.
  Scalar: hswish Relu and T-3 Copy, sigmoids (AHE with broadcast width-2
          trick keeps the DVE multiply at 2x), store DMA triggers.
  DMA:    1 contiguous 1 MB load (sync ring) + 1 MB store (scalar ring)
          per slice; channel shuffle + fp32 conversion on the host.
"""

import numpy as np

import concourse.bass as bass
import concourse.bacc as bacc
import concourse.mybir as mybir
import concourse.tile as tile
from concourse.bass_utils import run_bass_kernel_spmd

F32 = mybir.dt.float32
F16 = mybir.dt.float16

N_CORES = 8
NB = 2          # batches per core
C = 512
G = 4           # groups (of channels, in the model)
CG = 128        # channels per group
H = 64
W = 64
HW = H * W
S = NB * G      # slices per core
GRP = 2         # slices per partition-batched group
MIP = 16        # conv1 output channels
J = 8           # pooling octave width
EPS = 1e-5

_NC_CACHE = None


def _build_bass():
    nc = bacc.Bacc(None, target_bir_lowering=False)

    x_d = nc.dram_tensor("x", [S, CG, HW], F16, kind="ExternalInput")
    w1t_d = nc.dram_tensor("w1t", [CG, MIP], F16, kind="ExternalInput")
    whw4_d = nc.dram_tensor("whw4", [CG, 2 * CG], F16, kind="ExternalInput")
    bact4_d = nc.dram_tensor("bact4", [CG, 1], F32, kind="ExternalInput")
    bhw_d = nc.dram_tensor("bhw", [CG, 2], F32, kind="ExternalInput")
    out_d = nc.dram_tensor("out", [S, CG, HW], F16, kind="ExternalOutput")

    Relu = mybir.ActivationFunctionType.Relu
    Copy = mybir.ActivationFunctionType.Copy
    Sigmoid = mybir.ActivationFunctionType.Sigmoid
    ADD = mybir.AluOpType.add
    MIN = mybir.AluOpType.min
    MULT = mybir.AluOpType.mult

    with tile.TileContext(nc) as tc:
        with (
            tc.tile_pool(name="consts", bufs=1) as consts,
            tc.tile_pool(name="xp", bufs=8) as xp,
            tc.tile_pool(name="op", bufs=4) as op,
            tc.tile_pool(name="ps", bufs=1, space="PSUM") as ps,
            tc.tile_pool(name="sm", bufs=2) as sm,
        ):
            # consts for the conv path on the sync ring FIRST (ahead of the
            # x loads) so the PE warmup below can start ~9.5us in.
            w1t = consts.tile([CG, MIP], F16)
            nc.sync.dma_start(out=w1t, in_=w1t_d[:])
            whw4 = consts.tile([CG, 2 * CG], F16)
            nc.sync.dma_start(out=whw4, in_=whw4_d[:])
            bact4 = consts.tile([CG, 1], F32)
            nc.scalar.dma_start(out=bact4, in_=bact4_d[:])
            bhw = consts.tile([CG, 2], F32)
            nc.scalar.dma_start(out=bhw, in_=bhw_d[:])
            bh = bhw[:, 0:1]
            bw = bhw[:, 1:2]

            # ---- HAM warmup: ~3.4us of junk matmuls into P2 buffer 0 while
            # the first x load is still in flight; slice 0's octaves then
            # overwrite the region (start=True resets the accumulation).
            P2w = ps.tile([CG, 2, HW // J], F32, name="P2", tag="P2", bufs=2)
            for k in range(16):
                nc.tensor.matmul(
                    P2w[0:MIP, 0:1, 0 : 2 * CG],
                    w1t,
                    whw4,
                    start=True,
                    stop=True,
                    tile_position=(0, 0),
                )

            for g in range(S // GRP):
                # ---- conv1+pooling octaves for 2 slices into one PSUM tile.
                # band b=64i: h-part at [b:b+16, 0, (h j)] (j = w octave),
                # w-part at [b:b+16, 1, (j w)] (j = h octave, flat moving).
                P2 = ps.tile([CG, 2, HW // J], F32, name="P2", tag="P2", bufs=2)
                Xs = []
                for i in range(GRP):
                    s = GRP * g + i
                    b = 64 * i
                    X = xp.tile([CG, HW], F16, name="X")
                    nc.sync.dma_start(out=X, in_=x_d[s])
                    Xs.append(X)
                    Xr = X.rearrange("p (h w) -> p h w", h=H)
                    for k in range(HW // (H * J)):
                        nc.tensor.matmul(
                            P2[b : b + MIP, 0:1, :],
                            w1t,
                            Xr[:, :, J * k : J * (k + 1)],
                            start=(k == 0),
                            stop=(k == HW // (H * J) - 1),
                            tile_position=(0, b),
                        )
                    for k in range(HW // (W * J)):
                        nc.tensor.matmul(
                            P2[b : b + MIP, 1:2, :],
                            w1t,
                            X[:, J * W * k : J * W * (k + 1)],
                            start=(k == 0),
                            stop=(k == HW // (W * J) - 1),
                            tile_position=(0, b),
                        )

                # ---- skinny chain at high priority: fold + hswish + attention
                with tc.high_priority():
                    Y2 = sm.tile([CG, 2, H], F32, name="Y2", tag="Y2")
                    nc.vector.tensor_reduce(
                        out=Y2[:, 0:1, :],
                        in_=P2[:, 0:1, :].rearrange("p d (h j) -> p d h j", j=J),
                        axis=mybir.AxisListType.X,
                        op=ADD,
                    )
                    nc.vector.tensor_reduce(
                        out=Y2[:, 1:2, :],
                        in_=P2[:, 1:2, :].rearrange("p d (j w) -> p d w j", j=J),
                        axis=mybir.AxisListType.X,
                        op=ADD,
                    )
                    # hswish: HS = min(T,6) * (T-3), T = relu(Y + b1eff + 3)
                    T2 = sm.tile([CG, 2, H], F32, name="T2", tag="T2")
                    nc.scalar.activation(out=T2, in_=Y2, func=Relu, bias=bact4)
                    T3 = sm.tile([CG, 2, H], F32, name="T3", tag="T3")
                    nc.scalar.activation(out=T3, in_=T2, func=Copy, bias=-3.0)
                    HS2 = sm.tile([CG, 2, H], F16, name="HS2", tag="HS2")
                    nc.vector.scalar_tensor_tensor(
                        out=HS2, in0=T2, scalar=6.0, in1=T3, op0=MIN, op1=MULT
                    )

                    # attention logits: per slice, K=16 contraction on row
                    # tile b; each slice's logits in their own 2KB PSUM bank
                    # (concurrent row tiles must not share a bank).
                    APs2 = ps.tile([CG, GRP, 512], F32, name="APs2", tag="APs2", bufs=2)
                    for i in range(GRP):
                        b = 64 * i
                        nc.tensor.matmul(
                            APs2[:, i : i + 1, 0:H],
                            whw4[b : b + MIP, 0:CG],
                            HS2[b : b + MIP, 0:1, :],
                            start=True,
                            stop=True,
                            tile_position=(b, 0),
                        )
                        nc.tensor.matmul(
                            APs2[:, i : i + 1, H : 2 * H],
                            whw4[b : b + MIP, CG:],
                            HS2[b : b + MIP, 1:2, :],
                            start=True,
                            stop=True,
                            tile_position=(b, 0),
                        )

                    # batched sigmoids; AHE materialized at width TWO only
                    # (stride-0 middle dim keeps the DVE multiply at 2x rate)
                    AHE2 = sm.tile([CG, GRP, H, 2], F16, name="AHE2", tag="AHE2")
                    nc.scalar.activation(
                        out=AHE2,
                        in_=APs2[:, :, 0:H].unsqueeze(3).broadcast_to([CG, GRP, H, 2]),
                        func=Sigmoid,
                        bias=bh,
                    )
                    AW2 = sm.tile([CG, GRP, W], F16, name="AW2", tag="AW2")
                    nc.scalar.activation(
                        out=AW2, in_=APs2[:, :, H : 2 * H], func=Sigmoid, bias=bw
                    )

                # ---- out = x * a_h_exp * a_w  (both 2x-rate on DVE)
                for i in range(GRP):
                    s = GRP * g + i
                    X = Xs[i]
                    OUT = op.tile([CG, HW], F16, name="OUT")
                    OUTr = OUT.rearrange("p (h w) -> p h w", h=H)
                    Xr4 = X.rearrange("p (h r two) -> p h r two", h=H, two=2)
                    OUTr4 = OUT.rearrange("p (h r two) -> p h r two", h=H, two=2)
                    ahe_b = AHE2[:, i].unsqueeze(2).broadcast_to([CG, H, W // 2, 2])
                    aw_b = AW2[:, i].unsqueeze(1).broadcast_to([CG, H, W])
                    nc.vector.tensor_tensor(out=OUTr4, in0=Xr4, in1=ahe_b, op=MULT)
                    nc.vector.tensor_tensor(out=OUTr, in0=OUTr, in1=aw_b, op=MULT)
                    nc.scalar.dma_start(out=out_d[s], in_=OUT)

    nc.finalize()
    return nc


def _get_nc():
    global _NC_CACHE
    if _NC_CACHE is None:
        _NC_CACHE = _build_bass()
    return _NC_CACHE


def _prep_weights(W1, b1, gamma, beta, mean, var, Wh, bh, Ww, bw):
    W1 = np.asarray(W1, np.float64)
    b1 = np.asarray(b1, np.float64)
    gamma = np.asarray(gamma, np.float64)
    beta = np.asarray(beta, np.float64)
    mean = np.asarray(mean, np.float64)
    var = np.asarray(var, np.float64)
    Wh = np.asarray(Wh, np.float64)
    Ww = np.asarray(Ww, np.float64)
    bh = np.asarray(bh, np.float64)
    bw = np.asarray(bw, np.float64)

    scale = gamma / np.sqrt(var + EPS)                    # (MIP,)
    w1eff = (W1 * scale[:, None]) / float(W)              # (MIP, CG); mean 1/64
    b1eff = scale * (b1 - mean) + beta                    # (MIP,)

    w1t = np.ascontiguousarray(w1eff.T.astype(np.float16))            # (CG, MIP)
    whw = np.concatenate([(Wh / 6.0).T, (Ww / 6.0).T], axis=1)        # (MIP, 2CG)
    whw4 = np.zeros((CG, 2 * CG), np.float16)
    bact4 = np.zeros((CG, 1), np.float32)
    for b in range(0, CG, 32):
        whw4[b : b + MIP] = whw.astype(np.float16)
        bact4[b : b + MIP, 0] = (b1eff + 3.0).astype(np.float32)
    bhw = np.ascontiguousarray(
        np.stack([bh, bw], axis=1).astype(np.float32)
    )                                                     # (CG, 2)
    return w1t, whw4, bact4, bhw


def run(inputs: dict, trace: bool = False):
    """Run on 8 NeuronCores. Returns (out [16,512,64,64] fp32, results)."""
    x = np.asarray(inputs["x"], dtype=np.float32)
    n = x.shape[0]
    assert x.shape == (n, C, H, W) and n == N_CORES * NB, x.shape

    w1t, whw4, bact4, bhw = _prep_weights(
        inputs["W1"], inputs["b1"], inputs["gamma"], inputs["beta"],
        inputs["mean"], inputs["var"], inputs["Wh"], inputs["bh"],
        inputs["Ww"], inputs["bw"],
    )

    # fp16, pre-sliced per core: [core, slice(b,g), 128, 4096]
    x16 = np.ascontiguousarray(
        x.astype(np.float16).reshape(N_CORES, S, CG, HW)
    )

    nc = _get_nc()
    core_ids = list(range(N_CORES))
    in_maps = []
    for k in core_ids:
        in_maps.append(
            {
                "x": x16[k],
                "w1t": w1t,
                "whw4": whw4,
                "bact4": bact4,
                "bhw": bhw,
            }
        )

    res = run_bass_kernel_spmd(nc, in_maps, core_ids, trace=trace)
    out16 = np.stack([res.results[k]["out"] for k in core_ids])  # (8,8,128,HW)
    # group-major == natural channel order; then apply the channel shuffle
    # c' = (c % 4) * 128 + c // 4 on the host, with the fp16->fp32 upcast.
    nat = out16.astype(np.float32).reshape(n, C, H, W)
    out = np.ascontiguousarray(
        nat.reshape(n, CG, G, H, W).transpose(0, 2, 1, 3, 4).reshape(n, C, H, W)
    )
    return out, res


def kernel(**inputs) -> np.ndarray:
    out, _ = run(inputs, trace=False)
    return out


def exec_time_ns(res):
    return res.exec_time_ns
